# revision 3
# baseline (speedup 1.0000x reference)
"""Cross-attention kernel for Trainium2, data-parallel over batch on 8 NeuronCores.

Per core (batch element b):
  q = x[b] @ Wq.T + bq ; k = c[b] @ Wk.T + bk ; v = c[b] @ Wv.T + bv
  out[b] = softmax(q @ k.T / sqrt(D)) @ v

Device layout (all matmul operands bf16, fp32 accumulation):
  phase 1: QT[e,s] = (Wq @ x.T + bq)/sqrt(D)  (sc-outer so the first matmul
           needs only wq + 1MB of x), KT[e,t] = Wk @ c.T + bk, V[t,e] = c @ Wv.T + bv
  phase 2: per 128-row tile of s: S = QT.T @ KT (psum), P = exp(S) + row sums
           via ACT, per-512-chunk eager 128x128 xbar transposes, O = P @ V,
           scaled by 1/rowsum on drain; final store split across queues.

All DRAM inputs are host-pre-tiled so every DMA moves >=2KB contiguous
per-partition lines (the naive [d, e]-sliced weight tiles produced 256B
lines that trickled at ~30GB/s and starved the PE).
"""

import numpy as np
import ml_dtypes

import concourse.bass as bass
import concourse.mybir as mybir
import concourse.tile as tile
from concourse import bacc
from concourse.bass_utils import run_bass_kernel_spmd

DIM = 1024
SEQ = 2048
B = 8
P = 128
DT = DIM // P        # 8 contraction tiles of 128
ST = SEQ // P        # 16 seq tiles of 128
KC = SEQ // 512      # 4 key chunks of 512
EC = DIM // 512      # 2 embed chunks of 512
F32 = mybir.dt.float32
BF16 = mybir.dt.bfloat16

_CACHED_NC = None


def build_nc():
    nc = bacc.Bacc(None, target_bir_lowering=False)

    # xc/cc: [p, sc, dt, s_lo] = x[sc*512+s_lo, dt*128+p]  (bf16, 8KB lines)
    xc = nc.declare_dram_parameter("xc", [P, KC, DT, 512], BF16, isOutput=False)
    cc = nc.declare_dram_parameter("cc", [P, KC, DT, 512], BF16, isOutput=False)
    # wq/wk tiles: [et, p, dt, j] = W[et*128+j, dt*128+p]  (2KB lines per tile)
    wqt = nc.declare_dram_parameter("wqt", [DT, P, DT * P], BF16, isOutput=False)
    wkt = nc.declare_dram_parameter("wkt", [DT, P, DT * P], BF16, isOutput=False)
    # wv: [p, dt, e] = Wv[e, dt*128+p]  (16KB lines)
    wvt = nc.declare_dram_parameter("wvt", [P, DT * DIM], BF16, isOutput=False)
    bqs = nc.declare_dram_parameter("bqs", [DT, P], F32, isOutput=False)
    bks = nc.declare_dram_parameter("bks", [DT, P], F32, isOutput=False)
    bvb = nc.declare_dram_parameter("bvb", [P, DIM], F32, isOutput=False)
    out = nc.declare_dram_parameter("out", [SEQ, DIM], F32, isOutput=True)

    wqt_r = wqt.rearrange("t p e -> p t e")
    wkt_r = wkt.rearrange("t p e -> p t e")
    out_r = out.rearrange("(t p) e -> p t e", p=P)

    with tile.TileContext(nc) as tc:
        with (
            tc.tile_pool(name="resid", bufs=1) as resid,
            tc.tile_pool(name="singles", bufs=1) as singles,
        ):
            qt_sb = resid.tile([P, DT, SEQ], BF16, tag="qt")
            kt_sb = resid.tile([P, DT, SEQ], BF16, tag="kt")
            v_sb = resid.tile([P, ST, DIM], BF16, tag="v")

            bq_sb = singles.tile([P, DT], F32, tag="bq")
            bk_sb = singles.tile([P, DT], F32, tag="bk")
            bv_sb = singles.tile([P, DIM], F32, tag="bv")

            # ---------------- phase 1: projections ----------------
            with (
                tc.tile_pool(name="acts", bufs=1) as acts,
                tc.tile_pool(name="wqpool", bufs=1) as wqpool,
                tc.tile_pool(name="wkpool", bufs=3) as wkpool,
                tc.tile_pool(name="wvpool", bufs=1) as wvpool,
                tc.tile_pool(name="warmps", bufs=1, space="PSUM") as warmps,
                tc.tile_pool(name="ppool", bufs=6, space="PSUM") as ppool,
            ):
                # Dummy matmuls on a zeroed tile keep the PE busy through the
                # input-DMA window so HAM never sees an idle->busy transition
                # (which costs a ~7-10us half-clock window).
                wsrc = acts.tile([P, 512], BF16, tag="warm")
                nc.vector.memset(wsrc, 0.0)
                wps = warmps.tile([P, 512], F32, tag="wps")
                NWARM = 10
                for i in range(NWARM):
                    nc.tensor.matmul(
                        wps, wsrc[:, 0:P], wsrc, start=(i == 0), stop=(i == NWARM - 1)
                    )

                xt_sb = acts.tile([P, KC, DT, 512], BF16, tag="xt")
                ct_sb = acts.tile([P, KC, DT, 512], BF16, tag="ct")
                wq_sb = wqpool.tile([P, DT, DT * P], BF16, tag="wq")
                wv_sb = wvpool.tile([P, DT * DIM], BF16, tag="wv")

                # DMA schedule. gpsimd (SWDGE, starts ~5us later than the HW
                # queues): biases then the big phase-1 tail (ct, wv, bvb).
                nc.gpsimd.dma_start(out=bq_sb, in_=bqs.rearrange("t p -> p t"))
                nc.gpsimd.dma_start(out=bk_sb, in_=bks.rearrange("t p -> p t"))
                nc.gpsimd.dma_start(out=ct_sb, in_=cc[:, :])
                nc.gpsimd.dma_start(out=wv_sb, in_=wvt[:, :])
                nc.gpsimd.dma_start(out=bv_sb, in_=bvb[:, :])
                # sync HWDGE: first weight tile, then x interleaved with the
                # wq tiles in exactly the order the sc-outer q-proj consumes.
                nc.sync.dma_start(out=wq_sb[:, 0], in_=wqt_r[:, 0])
                nc.sync.dma_start(out=xt_sb[:, 0, 0:4], in_=xc[:, 0, 0:4])
                nc.sync.dma_start(out=wq_sb[:, 2], in_=wqt_r[:, 2])
                nc.sync.dma_start(out=wq_sb[:, 3], in_=wqt_r[:, 3])
                nc.sync.dma_start(out=wq_sb[:, 4], in_=wqt_r[:, 4])
                nc.sync.dma_start(out=wq_sb[:, 6], in_=wqt_r[:, 6])
                nc.sync.dma_start(out=xt_sb[:, 1], in_=xc[:, 1])
                # scalar HWDGE: second half of x sc0, remaining wq, x tail.
                nc.scalar.dma_start(out=xt_sb[:, 0, 4:8], in_=xc[:, 0, 4:8])
                nc.scalar.dma_start(out=wq_sb[:, 1], in_=wqt_r[:, 1])
                nc.scalar.dma_start(out=wq_sb[:, 5], in_=wqt_r[:, 5])
                nc.scalar.dma_start(out=wq_sb[:, 7], in_=wqt_r[:, 7])
                nc.scalar.dma_start(out=xt_sb[:, 2], in_=xc[:, 2])
                nc.scalar.dma_start(out=xt_sb[:, 3], in_=xc[:, 3])

                # q projection, sc-outer: first matmul needs only wq[0] + x sc0.
                for sc in range(KC):
                    for et in range(DT):
                        ps = ppool.tile([P, 512], F32, tag="proj")
                        for dt in range(DT):
                            nc.tensor.matmul(
                                ps,
                                wq_sb[:, et, dt * P : (dt + 1) * P],
                                xt_sb[:, sc, dt],
                                start=(dt == 0),
                                stop=(dt == DT - 1),
                            )
                        nc.scalar.activation(
                            out=qt_sb[:, et, sc * 512 : (sc + 1) * 512],
                            in_=ps,
                            func=mybir.ActivationFunctionType.Identity,
                            bias=bq_sb[:, et : et + 1],
                            scale=1.0 / 32.0,
                        )

                # k projection, et-outer (ct fully resident by then), wk
                # tiles stream through a 3-deep pool on the sync queue.
                for et in range(DT):
                    wk_t = wkpool.tile([P, DT * P], BF16, tag="wk")
                    nc.sync.dma_start(out=wk_t, in_=wkt_r[:, et])
                    for sc in range(KC):
                        ps = ppool.tile([P, 512], F32, tag="proj")
                        for dt in range(DT):
                            nc.tensor.matmul(
                                ps,
                                wk_t[:, dt * P : (dt + 1) * P],
                                ct_sb[:, sc, dt],
                                start=(dt == 0),
                                stop=(dt == DT - 1),
                            )
                        nc.scalar.activation(
                            out=kt_sb[:, et, sc * 512 : (sc + 1) * 512],
                            in_=ps,
                            func=mybir.ActivationFunctionType.Identity,
                            bias=bk_sb[:, et : et + 1],
                            scale=1.0,
                        )

                # v projection: out[t128, e512], CT tiles stationary
                for tt in range(ST):
                    sc, j = tt // 4, tt % 4
                    for ec in range(EC):
                        ps = ppool.tile([P, 512], F32, tag="proj")
                        for dt in range(DT):
                            nc.tensor.matmul(
                                ps,
                                ct_sb[:, sc, dt, j * P : (j + 1) * P],
                                wv_sb[:, dt * DIM + ec * 512 : dt * DIM + (ec + 1) * 512],
                                start=(dt == 0),
                                stop=(dt == DT - 1),
                            )
                        nc.vector.tensor_add(
                            out=v_sb[:, tt, ec * 512 : (ec + 1) * 512],
                            in0=ps,
                            in1=bv_sb[:, ec * 512 : (ec + 1) * 512],
                        )

            # ---------------- phase 2: attention ----------------
            # Software-pipelined: S/exp/transpose for tile st is emitted
            # before O/store for tile st-1. Transposes are per-512-chunk so
            # the last O stage starts right after the last S matmul.
            with (
                tc.tile_pool(name="attn", bufs=3) as attn,
                tc.tile_pool(name="stats", bufs=4) as stats,
                tc.tile_pool(name="spsum", bufs=5, space="PSUM") as spsum,
                tc.tile_pool(name="opsum", bufs=3, space="PSUM") as opsum,
            ):
                def emit_s_stage(st):
                    p_sb = attn.tile([P, SEQ], BF16, tag="p")
                    pt_sb = attn.tile([P, ST, P], BF16, tag="pt")
                    sums = stats.tile([P, KC], F32, tag="sums")
                    for kc in range(KC):
                        sp = spsum.tile([P, 512], F32, tag="s")
                        for dt in range(DT):
                            nc.tensor.matmul(
                                sp,
                                qt_sb[:, dt, st * P : (st + 1) * P],
                                kt_sb[:, dt, kc * 512 : (kc + 1) * 512],
                                start=(dt == 0),
                                stop=(dt == DT - 1),
                            )
                        nc.scalar.activation(
                            out=p_sb[:, kc * 512 : (kc + 1) * 512],
                            in_=sp,
                            func=mybir.ActivationFunctionType.Exp,
                            accum_out=sums[:, kc : kc + 1],
                        )
                        # eager xbar transpose of the 4 just-exp'd 128x128
                        # blocks: pt[p, tt, f] = p_sb[f, tt*128 + p]
                        nc.sync.dma_start_transpose(
                            out=pt_sb[:, kc * 4 : (kc + 1) * 4],
                            in_=p_sb[:, kc * 512 : (kc + 1) * 512],
                        )
                    ssum = stats.tile([P, 1], F32, tag="ssum")
                    rsum = stats.tile([P, 1], F32, tag="rsum")
                    nc.vector.reduce_sum(out=ssum, in_=sums, axis=mybir.AxisListType.X)
                    nc.vector.reciprocal(out=rsum, in_=ssum)
                    return pt_sb, rsum

                def emit_o_stage(st, pt_sb, rsum):
                    last = st == ST - 1
                    o_sb = attn.tile([P, DIM], F32, tag="o")
                    for ec in range(EC):
                        op = opsum.tile([P, 512], F32, tag="o")
                        for tt in range(ST):
                            nc.tensor.matmul(
                                op,
                                pt_sb[:, tt],
                                v_sb[:, tt, ec * 512 : (ec + 1) * 512],
                                start=(tt == 0),
                                stop=(tt == ST - 1),
                            )
                        nc.vector.tensor_scalar_mul(
                            out=o_sb[:, ec * 512 : (ec + 1) * 512],
                            in0=op,
                            scalar1=rsum,
                        )
                        if not last:
                            nc.gpsimd.dma_start(
                                out=out_r[:, st, ec * 512 : (ec + 1) * 512],
                                in_=o_sb[:, ec * 512 : (ec + 1) * 512],
                            )
                    if last:
                        # final store is latency-critical: ec0 overlaps the
                        # ec1 matmuls on gpsimd; ec1 splits across the two
                        # HW queues (idle by now).
                        nc.gpsimd.dma_start(
                            out=out_r[:, st, 0:512], in_=o_sb[:, 0:512]
                        )
                        nc.sync.dma_start(
                            out=out_r[:, st, 512:768], in_=o_sb[:, 512:768]
                        )
                        nc.scalar.dma_start(
                            out=out_r[:, st, 768:1024], in_=o_sb[:, 768:1024]
                        )

                pending = None
                for st in range(ST):
                    cur = emit_s_stage(st)
                    if pending is not None:
                        emit_o_stage(st - 1, *pending)
                    pending = cur
                emit_o_stage(ST - 1, *pending)

    nc.compile()
    return nc


def prep_inputs(x, context, Wq, bq, Wk, bk, Wv, bv):
    """Host-side prep: pre-tiled bf16 activations/weights (contiguous >=2KB
    DMA lines), tiled fp32 biases. Returns per-core input maps."""
    bf = ml_dtypes.bfloat16

    def act_tiles(a):  # [2048, 1024] -> [128, 4, 8, 512]
        return np.ascontiguousarray(
            a.reshape(KC, 512, DT, P).transpose(3, 0, 2, 1)
        ).astype(bf)

    def w_tiles(w):  # [1024, 1024] -> [8, 128, 1024]; [et,p,dt*128+j]=W[et*128+j, dt*128+p]
        return np.ascontiguousarray(
            w.reshape(DT, P, DT, P).transpose(0, 3, 2, 1).reshape(DT, P, DT * P)
        ).astype(bf)

    wqt = w_tiles(np.asarray(Wq, dtype=np.float32))
    wkt = w_tiles(np.asarray(Wk, dtype=np.float32))
    # [p, dt*1024+e] = Wv[e, dt*128+p]
    wvt = np.ascontiguousarray(
        np.asarray(Wv, dtype=np.float32).reshape(DIM, DT, P).transpose(2, 1, 0).reshape(P, DT * DIM)
    ).astype(bf)
    bqs = (bq.astype(np.float32) / 32.0).reshape(DT, P)
    bks = bk.astype(np.float32).reshape(DT, P)
    bvb = np.ascontiguousarray(
        np.broadcast_to(bv.astype(np.float32), (P, DIM))
    )
    in_maps = []
    for b in range(B):
        in_maps.append(
            {
                "xc": act_tiles(x[b]),
                "cc": act_tiles(context[b]),
                "wqt": wqt,
                "wkt": wkt,
                "wvt": wvt,
                "bqs": bqs,
                "bks": bks,
                "bvb": bvb,
            }
        )
    return in_maps


def kernel(x, context, Wq, bq, Wk, bk, Wv, bv):
    global _CACHED_NC
    x = np.asarray(x, dtype=np.float32)
    context = np.asarray(context, dtype=np.float32)
    in_maps = prep_inputs(x, context, np.asarray(Wq), np.asarray(bq),
                          np.asarray(Wk), np.asarray(bk),
                          np.asarray(Wv), np.asarray(bv))
    if _CACHED_NC is None:
        _CACHED_NC = build_nc()
    nc = _CACHED_NC
    core_ids = list(range(B))
    res = run_bass_kernel_spmd(nc, in_maps, core_ids)
    return np.stack([res.results[i]["out"] for i in core_ids]).astype(np.float32)
